# revision 18
# baseline (speedup 1.0000x reference)
# MoE layer (8 experts, top-2) on 8 TRN2 NeuronCores.
#
# Strategy: expert parallelism (core e owns expert e), per the sharding hint.
#   * Host (control plane): computes gate routing decisions, dispatches
#     ("all-to-all") each token's row to the core(s) owning its top-2 experts,
#     and combines the per-expert partial outputs back into the full output.
#   * Device (data plane): for each core e, computes
#         yT = sigmoid(dlg) * ( W2[e].T @ gelu( W1[e].T @ xT ) )
#     where xT is the (C x CAP) gathered token block for expert e (transposed
#     so the contraction dim lives on SBUF partitions), and sigmoid(dlg) is
#     exactly the top-2 softmax weight for the owning expert
#     (softmax([a,b])[0] == sigmoid(a-b)).
#
# Dataflow is fully transposed (features on partitions, tokens on the moving
# free dim) so neither matmul needs an intermediate transpose:
#     phase 1:  hT(F x T)  = W1.T @ xT   (accumulate over C tiles)  -> gelu
#     phase 2:  yT(C x T)  = W2.T @ hT   (accumulate over F tiles)  -> * ce
# W1 stays resident in SBUF in bf16; W2 streams per token block (its reloads
# hide under the PE-bound compute); tokens stream in blocks of 512 (the
# PSUM-bank moving-dim limit for fp32 accumulation).

import math

import numpy as np
import ml_dtypes

import concourse.bass as bass
import concourse.mybir as mybir
import concourse.tile as tile
from concourse import bacc
from concourse.bass_utils import run_bass_kernel_spmd

C = 1024          # d_model
F = 4096          # d_ff
E = 8             # experts == cores
P = 128           # SBUF partitions
NTOK = 512        # moving-dim token block (one PSUM bank of fp32)
BF16 = mybir.dt.bfloat16
F32 = mybir.dt.float32

# Filled by kernel() on each call, for the test harness to inspect.
last_run_info: dict = {}

# NEFF-module memo: cap -> compiled Bass module (routing is deterministic in
# the inputs, so repeat calls reuse the same module and its cached NEFF).
_nc_cache: dict = {}


def _build_ffn(cap: int, act_fn=None, ntok: int = NTOK) -> bass.Bass:
    """Per-core expert-FFN kernel: yt = sigmoid(dlg) * (w2.T @ gelu(w1.T @ xt))."""
    if act_fn is None:
        act_fn = mybir.ActivationFunctionType.Gelu
    nc = bacc.Bacc()
    CO = C // P   # 8 c-tiles
    FO = F // P   # 32 f-tiles

    xt = nc.dram_tensor("xt", [C, cap], BF16, kind="ExternalInput")
    w1 = nc.dram_tensor("w1", [C, F], BF16, kind="ExternalInput")
    # w2 is host-rearranged to [fi, co, fo, cc] so each (co) chunk streams as
    # one contiguous 8 KiB-per-partition DMA.
    w2 = nc.dram_tensor("w2", [P, CO, FO, P], BF16, kind="ExternalInput")
    dlg = nc.dram_tensor("dlg", [P, cap], F32, kind="ExternalInput")
    yt = nc.dram_tensor("yt", [C, cap], F32, kind="ExternalOutput")

    xt_r = xt.rearrange("(co ci) t -> ci co t", ci=P)
    yt_r = yt.rearrange("(co ci) t -> ci co t", ci=P)
    w1_r = w1.rearrange("(co ci) f -> ci co f", ci=P)

    with tile.TileContext(nc) as tc:
        with (
            tc.tile_pool(name="wts", bufs=1) as wpool,
            tc.tile_pool(name="w2s", bufs=3) as w2pool,
            tc.tile_pool(name="xts", bufs=2) as xpool,
            tc.tile_pool(name="hts", bufs=1) as hpool,
            tc.tile_pool(name="ces", bufs=2) as cepool,
            tc.tile_pool(name="yts", bufs=3) as ypool,
            tc.tile_pool(name="ps", bufs=4, space="PSUM") as pspool,
        ):
            # Resident w1 (bf16, 64 KiB/partition); w2 streams per token block.
            w1_sb = wpool.tile([P, CO, F], BF16, tag="w1")
            for co in range(CO):
                nc.sync.dma_start(w1_sb[:, co, :], w1_r[:, co, :])

            nblk = (cap + ntok - 1) // ntok
            for b in range(nblk):
                t0 = b * ntok
                tn = min(ntok, cap - t0)

                xt_t = xpool.tile([P, CO, ntok], BF16, tag="xt")
                nc.sync.dma_start(xt_t[:, :, :tn], xt_r[:, :, t0 : t0 + tn])

                # Combine weight ce = sigmoid(dlg) = 0.5 * tanh(dlg / 2) + 0.5
                # (tanh shares an ACT table with gelu; sigmoid does not).
                dlg_t = cepool.tile([P, ntok], F32, tag="dlg")
                nc.sync.dma_start(dlg_t[:, :tn], dlg[:, t0 : t0 + tn])
                ce_t = cepool.tile([P, ntok], F32, tag="ce")
                nc.scalar.activation(
                    ce_t[:, :tn], dlg_t[:, :tn],
                    mybir.ActivationFunctionType.Tanh, scale=0.5,
                )
                nc.vector.tensor_scalar(
                    ce_t[:, :tn], ce_t[:, :tn], 0.5, 0.5,
                    mybir.AluOpType.mult, mybir.AluOpType.add,
                )

                # Phase 1: hT = gelu(W1.T @ xT) for this token block.
                ht_t = hpool.tile([P, FO, ntok], BF16, tag="ht")
                for fo in range(FO):
                    ps = pspool.tile([P, ntok], F32, tag="ps")
                    for co in range(CO):
                        nc.tensor.matmul(
                            ps[:, :tn],
                            w1_sb[:, co, fo * P : (fo + 1) * P],
                            xt_t[:, co, :tn],
                            start=(co == 0),
                            stop=(co == CO - 1),
                        )
                    nc.scalar.activation(ht_t[:, fo, :tn], ps[:, :tn], act_fn)

                # Phase 2: yT = ce * (W2.T @ hT).
                for co in range(CO):
                    w2_t = w2pool.tile([P, FO, P], BF16, tag="w2s")
                    nc.sync.dma_start(w2_t[:], w2[:, co, :, :])
                    ps2 = pspool.tile([P, ntok], F32, tag="ps")
                    for fo in range(FO):
                        nc.tensor.matmul(
                            ps2[:, :tn],
                            w2_t[:, fo, :],
                            ht_t[:, fo, :tn],
                            start=(fo == 0),
                            stop=(fo == FO - 1),
                        )
                    y_t = ypool.tile([P, ntok], F32, tag="y")
                    nc.vector.tensor_tensor(
                        y_t[:, :tn], ps2[:, :tn], ce_t[:, :tn],
                        mybir.AluOpType.mult,
                    )
                    nc.sync.dma_start(yt_r[:, co, t0 : t0 + tn], y_t[:, :tn])

    # bacc passes: register allocation, and crucially generate_event_semaphores,
    # which splits multi-wait sync conditions (HW allows 1 wait per instruction).
    nc.compile()

    # Guard: the Tile allocator believes SBUF is 224 KiB/partition (the ISA
    # constant), but exceeding ~192 KiB crashes the TRN2 exec unit. Keep a
    # hard ceiling so overflows fail at build time, not on silicon.
    hw = 0
    for alloc in nc.to_json()["functions"][0]["allocations"]:
        for ml in alloc.get("memorylocations") or []:
            if ml.get("type") == "SB":
                hw = max(hw, ml["addr"] + ml["dims"][1])
    assert hw <= 184 * 1024, f"SBUF high-water {hw / 1024:.1f} KiB exceeds 184 KiB"
    return nc


def _gate_jax_cpu(xf: np.ndarray, Wg: np.ndarray):
    """Reproduce the reference's gate bit-exactly: fp32 matmul + lax.top_k
    on the jax CPU backend (including its tie-breaking). Falls back to a
    numpy gate (correct except possibly on exact fp32 knife-edge ties) if
    jax is unavailable."""
    try:
        import jax

        cpu = jax.devices("cpu")[0]
        with jax.default_device(cpu):
            logits = jax.device_put(xf, cpu) @ jax.device_put(Wg, cpu)
            tv, ti = jax.lax.top_k(logits, 2)
            return np.asarray(ti), np.asarray(tv)
    except Exception:
        logits = xf @ Wg
        part = np.argpartition(-logits, 1, axis=1)[:, :2]
        pv = np.take_along_axis(logits, part, axis=1)
        order = np.argsort(-pv, axis=1, kind="stable")
        ti = np.take_along_axis(part, order, axis=1)
        tv = np.take_along_axis(logits, ti, axis=1)
        return ti, tv


def kernel(x, Wg, W1, W2):
    x = np.asarray(x, dtype=np.float32)
    Wg = np.asarray(Wg, dtype=np.float32)
    W1 = np.asarray(W1, dtype=np.float32)
    W2 = np.asarray(W2, dtype=np.float32)

    B, T, _ = x.shape
    N = B * T
    xf = x.reshape(N, C)

    # ---- Gate + routing (control plane) ----
    # Routing decisions are knife-edge sensitive: for this problem one token
    # has a 2.7e-7 gap between its 2nd and 3rd expert logits, smaller than
    # fp32 GEMM rounding differences between BLAS implementations. Compute
    # the gate with the same jax-on-CPU ops the reference uses so the top-2
    # selection matches it bit-for-bit.
    top2, tv = _gate_jax_cpu(xf, Wg)                        # (N, 2) ids / logits

    sels = []
    counts = []
    for e in range(E):
        sel = np.nonzero((top2 == e).any(axis=1))[0]
        sels.append(sel)
        counts.append(len(sel))
    cap = max(P, math.ceil(max(counts) / P) * P)

    # ---- Token dispatch (all-to-all equivalent) ----
    in_maps = []
    for e in range(E):
        sel = sels[e]
        cnt = len(sel)
        row = top2[sel]
        tvr = tv[sel]
        own = np.where(row[:, 0] == e, tvr[:, 0], tvr[:, 1])
        other = np.where(row[:, 0] == e, tvr[:, 1], tvr[:, 0])

        xt = np.zeros((C, cap), dtype=ml_dtypes.bfloat16)
        xt[:, :cnt] = xf[sel].T.astype(ml_dtypes.bfloat16)
        dlg = np.full((cap,), -60.0, dtype=np.float32)
        dlg[:cnt] = own - other
        dlg_b = np.ascontiguousarray(
            np.broadcast_to(dlg[None, :], (P, cap)), dtype=np.float32
        )
        w2h = np.ascontiguousarray(
            W2[e].reshape(F // P, P, C // P, P).transpose(1, 2, 0, 3)
        ).astype(ml_dtypes.bfloat16)
        in_maps.append(
            {
                "xt": xt,
                "w1": W1[e].astype(ml_dtypes.bfloat16),
                "w2": w2h,
                "dlg": dlg_b,
            }
        )

    # ---- Expert FFN on the 8 NeuronCores ----
    nc = _nc_cache.get(cap)
    if nc is None:
        nc = _nc_cache[cap] = _build_ffn(cap)
    res = run_bass_kernel_spmd(nc, in_maps, core_ids=list(range(E)))

    global last_run_info
    last_run_info = {
        "cap": cap,
        "counts": counts,
        "exec_time_ns": res.exec_time_ns,
        "mean_exec_time_ns": res.mean_exec_time_ns,
        "instructions_and_trace": res.instructions_and_trace,
        "profile_json": res.profile_json,
    }

    # ---- Combine (weighted scatter-add) ----
    out = np.zeros((N, C), dtype=np.float32)
    for e in range(E):
        sel = sels[e]
        out[sel] += res.results[e]["yt"][:, : len(sel)].T
    return out.reshape(B, T, C)


# revision 19
# speedup vs baseline: 1.0292x; 1.0292x over previous
# MoE layer (8 experts, top-2) on 8 TRN2 NeuronCores.
#
# Strategy: expert parallelism (core e owns expert e), per the sharding hint.
#   * Host (control plane): computes gate routing decisions, dispatches
#     ("all-to-all") each token's row to the core(s) owning its top-2 experts,
#     and combines the per-expert partial outputs back into the full output.
#   * Device (data plane): for each core e, computes
#         yT = sigmoid(dlg) * ( W2[e].T @ gelu( W1[e].T @ xT ) )
#     where xT is the (C x CAP) gathered token block for expert e (transposed
#     so the contraction dim lives on SBUF partitions), and sigmoid(dlg) is
#     exactly the top-2 softmax weight for the owning expert
#     (softmax([a,b])[0] == sigmoid(a-b)).
#
# Dataflow is fully transposed (features on partitions, tokens on the moving
# free dim) so neither matmul needs an intermediate transpose:
#     phase 1:  hT(F x T)  = W1.T @ xT   (accumulate over C tiles)  -> gelu
#     phase 2:  yT(C x T)  = W2.T @ hT   (accumulate over F tiles)  -> * ce
# W1 stays resident in SBUF in bf16; W2 streams per token block (its reloads
# hide under the PE-bound compute); tokens stream in blocks of 512 (the
# PSUM-bank moving-dim limit for fp32 accumulation).

import math

import numpy as np
import ml_dtypes

import concourse.bass as bass
import concourse.mybir as mybir
import concourse.tile as tile
from concourse import bacc
from concourse.bass_utils import run_bass_kernel_spmd

C = 1024          # d_model
F = 4096          # d_ff
E = 8             # experts == cores
P = 128           # SBUF partitions
NTOK = 512        # moving-dim token block (one PSUM bank of fp32)
BF16 = mybir.dt.bfloat16
F32 = mybir.dt.float32

# Filled by kernel() on each call, for the test harness to inspect.
last_run_info: dict = {}

# NEFF-module memo: cap -> compiled Bass module (routing is deterministic in
# the inputs, so repeat calls reuse the same module and its cached NEFF).
_nc_cache: dict = {}


def _build_ffn(cap: int, act_fn=None, ntok: int = NTOK) -> bass.Bass:
    """Per-core expert-FFN kernel: yt = sigmoid(dlg) * (w2.T @ gelu(w1.T @ xt))."""
    if act_fn is None:
        act_fn = mybir.ActivationFunctionType.Gelu
    nc = bacc.Bacc()
    CO = C // P   # 8 c-tiles
    FO = F // P   # 32 f-tiles

    xt = nc.dram_tensor("xt", [C, cap], BF16, kind="ExternalInput")
    w1 = nc.dram_tensor("w1", [C, F], BF16, kind="ExternalInput")
    # w2 is host-rearranged to [fi, co, fo, cc] so each (co) chunk streams as
    # one contiguous 8 KiB-per-partition DMA.
    w2 = nc.dram_tensor("w2", [P, CO, FO, P], BF16, kind="ExternalInput")
    dlg = nc.dram_tensor("dlg", [P, cap], F32, kind="ExternalInput")
    yt = nc.dram_tensor("yt", [C, cap], F32, kind="ExternalOutput")

    xt_r = xt.rearrange("(co ci) t -> ci co t", ci=P)
    yt_r = yt.rearrange("(co ci) t -> ci co t", ci=P)
    w1_r = w1.rearrange("(co ci) f -> ci co f", ci=P)

    with tile.TileContext(nc) as tc:
        with (
            tc.tile_pool(name="wts", bufs=1) as wpool,
            tc.tile_pool(name="w2s", bufs=3) as w2pool,
            tc.tile_pool(name="xts", bufs=2) as xpool,
            tc.tile_pool(name="hts", bufs=1) as hpool,
            tc.tile_pool(name="ces", bufs=2) as cepool,
            tc.tile_pool(name="yts", bufs=3) as ypool,
            tc.tile_pool(name="ps", bufs=4, space="PSUM") as pspool,
        ):
            # Block 0's token DMAs are issued BEFORE the w1 load: the DMA
            # queue is FIFO, and the first matmul needs xt — queueing 8 MiB
            # of w1 ahead of it costs a ~23 us PE ramp (measured in the
            # cost-model timeline).
            xt0 = xpool.tile([P, CO, ntok], BF16, tag="xt")
            nc.sync.dma_start(xt0[:, :, : min(ntok, cap)], xt_r[:, :, : min(ntok, cap)])
            dlg0 = cepool.tile([P, ntok], F32, tag="dlg")
            nc.sync.dma_start(dlg0[:, : min(ntok, cap)], dlg[:, : min(ntok, cap)])

            # Resident w1 (bf16, 64 KiB/partition), loaded in f-major chunks
            # so phase 1's fo-th psum group only waits for the chunk covering
            # it, not the whole 8 MiB. w2 streams per token block.
            w1_sb = wpool.tile([P, CO, F], BF16, tag="w1")
            FCH = 512
            for f0 in range(0, F, FCH):
                for co in range(CO):
                    nc.sync.dma_start(
                        w1_sb[:, co, f0 : f0 + FCH], w1_r[:, co, f0 : f0 + FCH]
                    )

            nblk = (cap + ntok - 1) // ntok
            for b in range(nblk):
                t0 = b * ntok
                tn = min(ntok, cap - t0)

                if b == 0:
                    xt_t, dlg_t = xt0, dlg0
                else:
                    xt_t = xpool.tile([P, CO, ntok], BF16, tag="xt")
                    nc.sync.dma_start(xt_t[:, :, :tn], xt_r[:, :, t0 : t0 + tn])
                    # Combine weight ce = sigmoid(dlg) = 0.5*tanh(dlg/2) + 0.5
                    # (tanh shares an ACT table with gelu; sigmoid does not).
                    dlg_t = cepool.tile([P, ntok], F32, tag="dlg")
                    nc.sync.dma_start(dlg_t[:, :tn], dlg[:, t0 : t0 + tn])
                ce_t = cepool.tile([P, ntok], F32, tag="ce")
                nc.scalar.activation(
                    ce_t[:, :tn], dlg_t[:, :tn],
                    mybir.ActivationFunctionType.Tanh, scale=0.5,
                )
                nc.vector.tensor_scalar(
                    ce_t[:, :tn], ce_t[:, :tn], 0.5, 0.5,
                    mybir.AluOpType.mult, mybir.AluOpType.add,
                )

                # Phase 1: hT = gelu(W1.T @ xT) for this token block.
                ht_t = hpool.tile([P, FO, ntok], BF16, tag="ht")
                for fo in range(FO):
                    ps = pspool.tile([P, ntok], F32, tag="ps")
                    for co in range(CO):
                        nc.tensor.matmul(
                            ps[:, :tn],
                            w1_sb[:, co, fo * P : (fo + 1) * P],
                            xt_t[:, co, :tn],
                            start=(co == 0),
                            stop=(co == CO - 1),
                        )
                    nc.scalar.activation(ht_t[:, fo, :tn], ps[:, :tn], act_fn)

                # Phase 2: yT = ce * (W2.T @ hT).
                for co in range(CO):
                    w2_t = w2pool.tile([P, FO, P], BF16, tag="w2s")
                    nc.sync.dma_start(w2_t[:], w2[:, co, :, :])
                    ps2 = pspool.tile([P, ntok], F32, tag="ps")
                    for fo in range(FO):
                        nc.tensor.matmul(
                            ps2[:, :tn],
                            w2_t[:, fo, :],
                            ht_t[:, fo, :tn],
                            start=(fo == 0),
                            stop=(fo == FO - 1),
                        )
                    y_t = ypool.tile([P, ntok], F32, tag="y")
                    nc.vector.tensor_tensor(
                        y_t[:, :tn], ps2[:, :tn], ce_t[:, :tn],
                        mybir.AluOpType.mult,
                    )
                    nc.sync.dma_start(yt_r[:, co, t0 : t0 + tn], y_t[:, :tn])

    # bacc passes: register allocation, and crucially generate_event_semaphores,
    # which splits multi-wait sync conditions (HW allows 1 wait per instruction).
    nc.compile()

    # Guard: the Tile allocator believes SBUF is 224 KiB/partition (the ISA
    # constant), but exceeding ~192 KiB crashes the TRN2 exec unit. Keep a
    # hard ceiling so overflows fail at build time, not on silicon.
    hw = 0
    for alloc in nc.to_json()["functions"][0]["allocations"]:
        for ml in alloc.get("memorylocations") or []:
            if ml.get("type") == "SB":
                hw = max(hw, ml["addr"] + ml["dims"][1])
    assert hw <= 184 * 1024, f"SBUF high-water {hw / 1024:.1f} KiB exceeds 184 KiB"
    return nc


def _gate_jax_cpu(xf: np.ndarray, Wg: np.ndarray):
    """Reproduce the reference's gate bit-exactly: fp32 matmul + lax.top_k
    on the jax CPU backend (including its tie-breaking). Falls back to a
    numpy gate (correct except possibly on exact fp32 knife-edge ties) if
    jax is unavailable."""
    try:
        import jax

        cpu = jax.devices("cpu")[0]
        with jax.default_device(cpu):
            logits = jax.device_put(xf, cpu) @ jax.device_put(Wg, cpu)
            tv, ti = jax.lax.top_k(logits, 2)
            return np.asarray(ti), np.asarray(tv)
    except Exception:
        logits = xf @ Wg
        part = np.argpartition(-logits, 1, axis=1)[:, :2]
        pv = np.take_along_axis(logits, part, axis=1)
        order = np.argsort(-pv, axis=1, kind="stable")
        ti = np.take_along_axis(part, order, axis=1)
        tv = np.take_along_axis(logits, ti, axis=1)
        return ti, tv


def kernel(x, Wg, W1, W2):
    x = np.asarray(x, dtype=np.float32)
    Wg = np.asarray(Wg, dtype=np.float32)
    W1 = np.asarray(W1, dtype=np.float32)
    W2 = np.asarray(W2, dtype=np.float32)

    B, T, _ = x.shape
    N = B * T
    xf = x.reshape(N, C)

    # ---- Gate + routing (control plane) ----
    # Routing decisions are knife-edge sensitive: for this problem one token
    # has a 2.7e-7 gap between its 2nd and 3rd expert logits, smaller than
    # fp32 GEMM rounding differences between BLAS implementations. Compute
    # the gate with the same jax-on-CPU ops the reference uses so the top-2
    # selection matches it bit-for-bit.
    top2, tv = _gate_jax_cpu(xf, Wg)                        # (N, 2) ids / logits

    sels = []
    counts = []
    for e in range(E):
        sel = np.nonzero((top2 == e).any(axis=1))[0]
        sels.append(sel)
        counts.append(len(sel))
    cap = max(P, math.ceil(max(counts) / P) * P)

    # ---- Token dispatch (all-to-all equivalent) ----
    in_maps = []
    for e in range(E):
        sel = sels[e]
        cnt = len(sel)
        row = top2[sel]
        tvr = tv[sel]
        own = np.where(row[:, 0] == e, tvr[:, 0], tvr[:, 1])
        other = np.where(row[:, 0] == e, tvr[:, 1], tvr[:, 0])

        xt = np.zeros((C, cap), dtype=ml_dtypes.bfloat16)
        xt[:, :cnt] = xf[sel].T.astype(ml_dtypes.bfloat16)
        dlg = np.full((cap,), -60.0, dtype=np.float32)
        dlg[:cnt] = own - other
        dlg_b = np.ascontiguousarray(
            np.broadcast_to(dlg[None, :], (P, cap)), dtype=np.float32
        )
        w2h = np.ascontiguousarray(
            W2[e].reshape(F // P, P, C // P, P).transpose(1, 2, 0, 3)
        ).astype(ml_dtypes.bfloat16)
        in_maps.append(
            {
                "xt": xt,
                "w1": W1[e].astype(ml_dtypes.bfloat16),
                "w2": w2h,
                "dlg": dlg_b,
            }
        )

    # ---- Expert FFN on the 8 NeuronCores ----
    nc = _nc_cache.get(cap)
    if nc is None:
        nc = _nc_cache[cap] = _build_ffn(cap)
    res = run_bass_kernel_spmd(nc, in_maps, core_ids=list(range(E)))

    global last_run_info
    last_run_info = {
        "cap": cap,
        "counts": counts,
        "exec_time_ns": res.exec_time_ns,
        "mean_exec_time_ns": res.mean_exec_time_ns,
        "instructions_and_trace": res.instructions_and_trace,
        "profile_json": res.profile_json,
    }

    # ---- Combine (weighted scatter-add) ----
    out = np.zeros((N, C), dtype=np.float32)
    for e in range(E):
        sel = sels[e]
        out[sel] += res.results[e]["yt"][:, : len(sel)].T
    return out.reshape(B, T, C)


# revision 21
# speedup vs baseline: 1.0300x; 1.0008x over previous
# MoE layer (8 experts, top-2) on 8 TRN2 NeuronCores.
#
# Strategy: expert parallelism (core e owns expert e), per the sharding hint.
#   * Host (control plane): computes gate routing decisions, dispatches
#     ("all-to-all") each token's row to the core(s) owning its top-2 experts,
#     and combines the per-expert partial outputs back into the full output.
#   * Device (data plane): for each core e, computes
#         yT = sigmoid(dlg) * ( W2[e].T @ gelu( W1[e].T @ xT ) )
#     where xT is the (C x CAP) gathered token block for expert e (transposed
#     so the contraction dim lives on SBUF partitions), and sigmoid(dlg) is
#     exactly the top-2 softmax weight for the owning expert
#     (softmax([a,b])[0] == sigmoid(a-b)).
#
# Dataflow is fully transposed (features on partitions, tokens on the moving
# free dim) so neither matmul needs an intermediate transpose:
#     phase 1:  hT(F x T)  = W1.T @ xT   (accumulate over C tiles)  -> gelu
#     phase 2:  yT(C x T)  = W2.T @ hT   (accumulate over F tiles)  -> * ce
# W1 stays resident in SBUF in bf16; W2 streams per token block (its reloads
# hide under the PE-bound compute); tokens stream in blocks of 512 (the
# PSUM-bank moving-dim limit for fp32 accumulation).

import math

import numpy as np
import ml_dtypes

import concourse.bass as bass
import concourse.mybir as mybir
import concourse.tile as tile
from concourse import bacc
from concourse.bass_utils import run_bass_kernel_spmd

C = 1024          # d_model
F = 4096          # d_ff
E = 8             # experts == cores
P = 128           # SBUF partitions
NTOK = 512        # moving-dim token block (one PSUM bank of fp32)
BF16 = mybir.dt.bfloat16
F32 = mybir.dt.float32

# Filled by kernel() on each call, for the test harness to inspect.
last_run_info: dict = {}

# NEFF-module memo: cap -> compiled Bass module (routing is deterministic in
# the inputs, so repeat calls reuse the same module and its cached NEFF).
_nc_cache: dict = {}


def _build_ffn(cap: int, act_fn=None, ntok: int = NTOK) -> bass.Bass:
    """Per-core expert-FFN kernel: yt = sigmoid(dlg) * (w2.T @ gelu(w1.T @ xt))."""
    if act_fn is None:
        act_fn = mybir.ActivationFunctionType.Gelu
    nc = bacc.Bacc()
    CO = C // P   # 8 c-tiles
    FO = F // P   # 32 f-tiles

    xt = nc.dram_tensor("xt", [C, cap], BF16, kind="ExternalInput")
    w1 = nc.dram_tensor("w1", [C, F], BF16, kind="ExternalInput")
    # w2 is host-rearranged to [fi, co, fo, cc] so each (co) chunk streams as
    # one contiguous 8 KiB-per-partition DMA.
    w2 = nc.dram_tensor("w2", [P, CO, FO, P], BF16, kind="ExternalInput")
    dlg = nc.dram_tensor("dlg", [P, cap], F32, kind="ExternalInput")
    yt = nc.dram_tensor("yt", [C, cap], F32, kind="ExternalOutput")

    xt_r = xt.rearrange("(co ci) t -> ci co t", ci=P)
    yt_r = yt.rearrange("(co ci) t -> ci co t", ci=P)
    w1_r = w1.rearrange("(co ci) f -> ci co f", ci=P)

    with tile.TileContext(nc) as tc:
        with (
            tc.tile_pool(name="wts", bufs=1) as wpool,
            tc.tile_pool(name="w2s", bufs=3) as w2pool,
            tc.tile_pool(name="xts", bufs=2) as xpool,
            tc.tile_pool(name="hts", bufs=1) as hpool,
            tc.tile_pool(name="ces", bufs=2) as cepool,
            tc.tile_pool(name="yts", bufs=3) as ypool,
            tc.tile_pool(name="ps", bufs=4, space="PSUM") as pspool,
        ):
            # Block 0's token DMAs are issued BEFORE the w1 load: the DMA
            # queue is FIFO, and the first matmul needs xt — queueing 8 MiB
            # of w1 ahead of it costs a ~23 us PE ramp (measured in the
            # cost-model timeline).
            xt0 = xpool.tile([P, CO, ntok], BF16, tag="xt")
            nc.sync.dma_start(xt0[:, :, : min(ntok, cap)], xt_r[:, :, : min(ntok, cap)])

            # Resident w1 (bf16, 64 KiB/partition), loaded in f-major chunks
            # so phase 1's fo-th psum group only waits for the chunk covering
            # it, not the whole 8 MiB. w2 streams per token block.
            w1_sb = wpool.tile([P, CO, F], BF16, tag="w1")
            FCH = 512
            for f0 in range(0, F, FCH):
                for co in range(CO):
                    nc.sync.dma_start(
                        w1_sb[:, co, f0 : f0 + FCH], w1_r[:, co, f0 : f0 + FCH]
                    )

            nblk = (cap + ntok - 1) // ntok
            for b in range(nblk):
                t0 = b * ntok
                tn = min(ntok, cap - t0)

                if b == 0:
                    xt_t = xt0
                else:
                    xt_t = xpool.tile([P, CO, ntok], BF16, tag="xt")
                    nc.sync.dma_start(xt_t[:, :, :tn], xt_r[:, :, t0 : t0 + tn])
                # Combine weight ce = sigmoid(dlg) = 0.5*tanh(dlg/2) + 0.5
                # (tanh shares an ACT table with gelu; sigmoid does not).
                # dlg isn't needed until phase 2, so even block 0's load sits
                # after the w1 chunks without stalling anything.
                dlg_t = cepool.tile([P, ntok], F32, tag="dlg")
                nc.sync.dma_start(dlg_t[:, :tn], dlg[:, t0 : t0 + tn])
                ce_t = cepool.tile([P, ntok], F32, tag="ce")
                nc.scalar.activation(
                    ce_t[:, :tn], dlg_t[:, :tn],
                    mybir.ActivationFunctionType.Tanh, scale=0.5,
                )
                nc.vector.tensor_scalar(
                    ce_t[:, :tn], ce_t[:, :tn], 0.5, 0.5,
                    mybir.AluOpType.mult, mybir.AluOpType.add,
                )

                # Phase 1: hT = gelu(W1.T @ xT) for this token block.
                ht_t = hpool.tile([P, FO, ntok], BF16, tag="ht")
                for fo in range(FO):
                    ps = pspool.tile([P, ntok], F32, tag="ps")
                    for co in range(CO):
                        nc.tensor.matmul(
                            ps[:, :tn],
                            w1_sb[:, co, fo * P : (fo + 1) * P],
                            xt_t[:, co, :tn],
                            start=(co == 0),
                            stop=(co == CO - 1),
                        )
                    nc.scalar.activation(ht_t[:, fo, :tn], ps[:, :tn], act_fn)

                # Phase 2: yT = ce * (W2.T @ hT).
                for co in range(CO):
                    w2_t = w2pool.tile([P, FO, P], BF16, tag="w2s")
                    nc.sync.dma_start(w2_t[:], w2[:, co, :, :])
                    ps2 = pspool.tile([P, ntok], F32, tag="ps")
                    for fo in range(FO):
                        nc.tensor.matmul(
                            ps2[:, :tn],
                            w2_t[:, fo, :],
                            ht_t[:, fo, :tn],
                            start=(fo == 0),
                            stop=(fo == FO - 1),
                        )
                    y_t = ypool.tile([P, ntok], F32, tag="y")
                    nc.vector.tensor_tensor(
                        y_t[:, :tn], ps2[:, :tn], ce_t[:, :tn],
                        mybir.AluOpType.mult,
                    )
                    nc.sync.dma_start(yt_r[:, co, t0 : t0 + tn], y_t[:, :tn])

    # bacc passes: register allocation, and crucially generate_event_semaphores,
    # which splits multi-wait sync conditions (HW allows 1 wait per instruction).
    nc.compile()

    # Guard: the Tile allocator believes SBUF is 224 KiB/partition (the ISA
    # constant), but exceeding ~192 KiB crashes the TRN2 exec unit. Keep a
    # hard ceiling so overflows fail at build time, not on silicon.
    hw = 0
    for alloc in nc.to_json()["functions"][0]["allocations"]:
        for ml in alloc.get("memorylocations") or []:
            if ml.get("type") == "SB":
                hw = max(hw, ml["addr"] + ml["dims"][1])
    assert hw <= 184 * 1024, f"SBUF high-water {hw / 1024:.1f} KiB exceeds 184 KiB"
    return nc


def _gate_jax_cpu(xf: np.ndarray, Wg: np.ndarray):
    """Reproduce the reference's gate bit-exactly: fp32 matmul + lax.top_k
    on the jax CPU backend (including its tie-breaking). Falls back to a
    numpy gate (correct except possibly on exact fp32 knife-edge ties) if
    jax is unavailable."""
    try:
        import jax

        cpu = jax.devices("cpu")[0]
        with jax.default_device(cpu):
            logits = jax.device_put(xf, cpu) @ jax.device_put(Wg, cpu)
            tv, ti = jax.lax.top_k(logits, 2)
            return np.asarray(ti), np.asarray(tv)
    except Exception:
        logits = xf @ Wg
        part = np.argpartition(-logits, 1, axis=1)[:, :2]
        pv = np.take_along_axis(logits, part, axis=1)
        order = np.argsort(-pv, axis=1, kind="stable")
        ti = np.take_along_axis(part, order, axis=1)
        tv = np.take_along_axis(logits, ti, axis=1)
        return ti, tv


def kernel(x, Wg, W1, W2):
    x = np.asarray(x, dtype=np.float32)
    Wg = np.asarray(Wg, dtype=np.float32)
    W1 = np.asarray(W1, dtype=np.float32)
    W2 = np.asarray(W2, dtype=np.float32)

    B, T, _ = x.shape
    N = B * T
    xf = x.reshape(N, C)

    # ---- Gate + routing (control plane) ----
    # Routing decisions are knife-edge sensitive: for this problem one token
    # has a 2.7e-7 gap between its 2nd and 3rd expert logits, smaller than
    # fp32 GEMM rounding differences between BLAS implementations. Compute
    # the gate with the same jax-on-CPU ops the reference uses so the top-2
    # selection matches it bit-for-bit.
    top2, tv = _gate_jax_cpu(xf, Wg)                        # (N, 2) ids / logits

    sels = []
    counts = []
    for e in range(E):
        sel = np.nonzero((top2 == e).any(axis=1))[0]
        sels.append(sel)
        counts.append(len(sel))
    cap = max(P, math.ceil(max(counts) / P) * P)

    # ---- Token dispatch (all-to-all equivalent) ----
    in_maps = []
    for e in range(E):
        sel = sels[e]
        cnt = len(sel)
        row = top2[sel]
        tvr = tv[sel]
        own = np.where(row[:, 0] == e, tvr[:, 0], tvr[:, 1])
        other = np.where(row[:, 0] == e, tvr[:, 1], tvr[:, 0])

        xt = np.zeros((C, cap), dtype=ml_dtypes.bfloat16)
        xt[:, :cnt] = xf[sel].T.astype(ml_dtypes.bfloat16)
        dlg = np.full((cap,), -60.0, dtype=np.float32)
        dlg[:cnt] = own - other
        dlg_b = np.ascontiguousarray(
            np.broadcast_to(dlg[None, :], (P, cap)), dtype=np.float32
        )
        w2h = np.ascontiguousarray(
            W2[e].reshape(F // P, P, C // P, P).transpose(1, 2, 0, 3)
        ).astype(ml_dtypes.bfloat16)
        in_maps.append(
            {
                "xt": xt,
                "w1": W1[e].astype(ml_dtypes.bfloat16),
                "w2": w2h,
                "dlg": dlg_b,
            }
        )

    # ---- Expert FFN on the 8 NeuronCores ----
    nc = _nc_cache.get(cap)
    if nc is None:
        nc = _nc_cache[cap] = _build_ffn(cap)
    res = run_bass_kernel_spmd(nc, in_maps, core_ids=list(range(E)))

    global last_run_info
    last_run_info = {
        "cap": cap,
        "counts": counts,
        "exec_time_ns": res.exec_time_ns,
        "mean_exec_time_ns": res.mean_exec_time_ns,
        "instructions_and_trace": res.instructions_and_trace,
        "profile_json": res.profile_json,
    }

    # ---- Combine (weighted scatter-add) ----
    out = np.zeros((N, C), dtype=np.float32)
    for e in range(E):
        sel = sels[e]
        out[sel] += res.results[e]["yt"][:, : len(sel)].T
    return out.reshape(B, T, C)


# revision 22
# speedup vs baseline: 1.0333x; 1.0033x over previous
# MoE layer (8 experts, top-2) on 8 TRN2 NeuronCores.
#
# Strategy: expert parallelism (core e owns expert e), per the sharding hint.
#   * Host (control plane): computes gate routing decisions, dispatches
#     ("all-to-all") each token's row to the core(s) owning its top-2 experts,
#     and combines the per-expert partial outputs back into the full output.
#   * Device (data plane): for each core e, computes
#         yT = sigmoid(dlg) * ( W2[e].T @ gelu( W1[e].T @ xT ) )
#     where xT is the (C x CAP) gathered token block for expert e (transposed
#     so the contraction dim lives on SBUF partitions), and sigmoid(dlg) is
#     exactly the top-2 softmax weight for the owning expert
#     (softmax([a,b])[0] == sigmoid(a-b)).
#
# Dataflow is fully transposed (features on partitions, tokens on the moving
# free dim) so neither matmul needs an intermediate transpose:
#     phase 1:  hT(F x T)  = W1.T @ xT   (accumulate over C tiles)  -> gelu
#     phase 2:  yT(C x T)  = W2.T @ hT   (accumulate over F tiles)  -> * ce
# W1 stays resident in SBUF in bf16; W2 streams per token block (its reloads
# hide under the PE-bound compute); tokens stream in blocks of 512 (the
# PSUM-bank moving-dim limit for fp32 accumulation).

import math

import numpy as np
import ml_dtypes

import concourse.bass as bass
import concourse.mybir as mybir
import concourse.tile as tile
from concourse import bacc
from concourse.bass_utils import run_bass_kernel_spmd

C = 1024          # d_model
F = 4096          # d_ff
E = 8             # experts == cores
P = 128           # SBUF partitions
NTOK = 512        # moving-dim token block (one PSUM bank of fp32)
BF16 = mybir.dt.bfloat16
F32 = mybir.dt.float32

# Filled by kernel() on each call, for the test harness to inspect.
last_run_info: dict = {}

# NEFF-module memo: cap -> compiled Bass module (routing is deterministic in
# the inputs, so repeat calls reuse the same module and its cached NEFF).
_nc_cache: dict = {}


def _build_ffn(cap: int, act_fn=None, ntok: int = NTOK) -> bass.Bass:
    """Per-core expert-FFN kernel: yt = sigmoid(dlg) * (w2.T @ gelu(w1.T @ xt))."""
    if act_fn is None:
        act_fn = mybir.ActivationFunctionType.Gelu
    nc = bacc.Bacc()
    CO = C // P   # 8 c-tiles
    FO = F // P   # 32 f-tiles

    xt = nc.dram_tensor("xt", [C, cap], BF16, kind="ExternalInput")
    w1 = nc.dram_tensor("w1", [C, F], BF16, kind="ExternalInput")
    # w2 is host-rearranged to [fi, co, fo, cc] so each (co) chunk streams as
    # one contiguous 8 KiB-per-partition DMA.
    w2 = nc.dram_tensor("w2", [P, CO, FO, P], BF16, kind="ExternalInput")
    dlg = nc.dram_tensor("dlg", [P, cap], F32, kind="ExternalInput")
    yt = nc.dram_tensor("yt", [C, cap], F32, kind="ExternalOutput")

    xt_r = xt.rearrange("(co ci) t -> ci co t", ci=P)
    yt_r = yt.rearrange("(co ci) t -> ci co t", ci=P)
    w1_r = w1.rearrange("(co ci) f -> ci co f", ci=P)

    with tile.TileContext(nc) as tc:
        with (
            tc.tile_pool(name="wts", bufs=1) as wpool,
            tc.tile_pool(name="w2s", bufs=3) as w2pool,
            tc.tile_pool(name="xts", bufs=2) as xpool,
            tc.tile_pool(name="hts", bufs=1) as hpool,
            tc.tile_pool(name="ces", bufs=2) as cepool,
            tc.tile_pool(name="yts", bufs=3) as ypool,
            tc.tile_pool(name="ps", bufs=4, space="PSUM") as pspool,
        ):
            # Block 0's token DMAs are issued BEFORE the w1 load: the DMA
            # queue is FIFO, and the first matmul needs xt — queueing 8 MiB
            # of w1 ahead of it costs a ~23 us PE ramp (measured in the
            # cost-model timeline).
            xt0 = xpool.tile([P, CO, ntok], BF16, tag="xt")
            nc.sync.dma_start(xt0[:, :, : min(ntok, cap)], xt_r[:, :, : min(ntok, cap)])

            # Resident w1 (bf16, 64 KiB/partition), loaded in f-major chunks
            # so phase 1's fo-th psum group only waits for the chunk covering
            # it, not the whole 8 MiB. w2 streams per token block.
            w1_sb = wpool.tile([P, CO, F], BF16, tag="w1")
            FCH = 512
            for f0 in range(0, F, FCH):
                for co in range(CO):
                    nc.sync.dma_start(
                        w1_sb[:, co, f0 : f0 + FCH], w1_r[:, co, f0 : f0 + FCH]
                    )

            nblk = (cap + ntok - 1) // ntok
            for b in range(nblk):
                t0 = b * ntok
                tn = min(ntok, cap - t0)

                if b == 0:
                    xt_t = xt0
                else:
                    xt_t = xpool.tile([P, CO, ntok], BF16, tag="xt")
                    nc.sync.dma_start(xt_t[:, :, :tn], xt_r[:, :, t0 : t0 + tn])
                # Combine weight ce = sigmoid(dlg) = 0.5*tanh(dlg/2) + 0.5
                # (tanh shares an ACT table with gelu; sigmoid does not).
                # dlg isn't needed until phase 2, so even block 0's load sits
                # after the w1 chunks without stalling anything.
                dlg_t = cepool.tile([P, ntok], F32, tag="dlg")
                nc.sync.dma_start(dlg_t[:, :tn], dlg[:, t0 : t0 + tn])
                ce_t = cepool.tile([P, ntok], F32, tag="ce")
                nc.scalar.activation(
                    ce_t[:, :tn], dlg_t[:, :tn],
                    mybir.ActivationFunctionType.Tanh, scale=0.5,
                )
                nc.vector.tensor_scalar(
                    ce_t[:, :tn], ce_t[:, :tn], 0.5, 0.5,
                    mybir.AluOpType.mult, mybir.AluOpType.add,
                )

                # Phase 1: hT = gelu(W1.T @ xT) for this token block.
                ht_t = hpool.tile([P, FO, ntok], BF16, tag="ht")
                for fo in range(FO):
                    ps = pspool.tile([P, ntok], F32, tag="ps")
                    for co in range(CO):
                        nc.tensor.matmul(
                            ps[:, :tn],
                            w1_sb[:, co, fo * P : (fo + 1) * P],
                            xt_t[:, co, :tn],
                            start=(co == 0),
                            stop=(co == CO - 1),
                        )
                    nc.scalar.activation(ht_t[:, fo, :tn], ps[:, :tn], act_fn)

                # Phase 2: yT = ce * (W2.T @ hT).
                for co in range(CO):
                    w2_t = w2pool.tile([P, FO, P], BF16, tag="w2s")
                    nc.sync.dma_start(w2_t[:], w2[:, co, :, :])
                    ps2 = pspool.tile([P, ntok], F32, tag="ps")
                    for fo in range(FO):
                        nc.tensor.matmul(
                            ps2[:, :tn],
                            w2_t[:, fo, :],
                            ht_t[:, fo, :tn],
                            start=(fo == 0),
                            stop=(fo == FO - 1),
                        )
                    y_t = ypool.tile([P, ntok], F32, tag="y")
                    nc.vector.tensor_tensor(
                        y_t[:, :tn], ps2[:, :tn], ce_t[:, :tn],
                        mybir.AluOpType.mult,
                    )
                    nc.sync.dma_start(yt_r[:, co, t0 : t0 + tn], y_t[:, :tn])

    # bacc passes: register allocation, and crucially generate_event_semaphores,
    # which splits multi-wait sync conditions (HW allows 1 wait per instruction).
    nc.compile()

    # Guard: the Tile allocator believes SBUF is 224 KiB/partition (the ISA
    # constant), but exceeding ~192 KiB crashes the TRN2 exec unit. Keep a
    # hard ceiling so overflows fail at build time, not on silicon.
    hw = 0
    for alloc in nc.to_json()["functions"][0]["allocations"]:
        for ml in alloc.get("memorylocations") or []:
            if ml.get("type") == "SB":
                hw = max(hw, ml["addr"] + ml["dims"][1])
    assert hw <= 184 * 1024, f"SBUF high-water {hw / 1024:.1f} KiB exceeds 184 KiB"
    return nc


def _gate_jax_cpu(xf: np.ndarray, Wg: np.ndarray):
    """Reproduce the reference's gate bit-exactly: fp32 matmul + lax.top_k
    on the jax CPU backend (including its tie-breaking). Falls back to a
    numpy gate (correct except possibly on exact fp32 knife-edge ties) if
    jax is unavailable."""
    try:
        import jax

        cpu = jax.devices("cpu")[0]
        with jax.default_device(cpu):
            logits = jax.device_put(xf, cpu) @ jax.device_put(Wg, cpu)
            tv, ti = jax.lax.top_k(logits, 2)
            return np.asarray(ti), np.asarray(tv)
    except Exception:
        logits = xf @ Wg
        part = np.argpartition(-logits, 1, axis=1)[:, :2]
        pv = np.take_along_axis(logits, part, axis=1)
        order = np.argsort(-pv, axis=1, kind="stable")
        ti = np.take_along_axis(part, order, axis=1)
        tv = np.take_along_axis(logits, ti, axis=1)
        return ti, tv


def kernel(x, Wg, W1, W2):
    x = np.asarray(x, dtype=np.float32)
    Wg = np.asarray(Wg, dtype=np.float32)
    W1 = np.asarray(W1, dtype=np.float32)
    W2 = np.asarray(W2, dtype=np.float32)

    B, T, _ = x.shape
    N = B * T
    xf = x.reshape(N, C)

    # ---- Gate + routing (control plane) ----
    # Routing decisions are knife-edge sensitive: for this problem one token
    # has a 2.7e-7 gap between its 2nd and 3rd expert logits, smaller than
    # fp32 GEMM rounding differences between BLAS implementations. Compute
    # the gate with the same jax-on-CPU ops the reference uses so the top-2
    # selection matches it bit-for-bit.
    top2, tv = _gate_jax_cpu(xf, Wg)                        # (N, 2) ids / logits

    sels = []
    counts = []
    for e in range(E):
        sel = np.nonzero((top2 == e).any(axis=1))[0]
        sels.append(sel)
        counts.append(len(sel))
    # cap needs no partition alignment — tokens are the free dim everywhere.
    # Round to even so bf16 rows stay 4-byte aligned.
    cap = max(NTOK, math.ceil(max(counts) / 2) * 2)

    # ---- Token dispatch (all-to-all equivalent) ----
    in_maps = []
    for e in range(E):
        sel = sels[e]
        cnt = len(sel)
        row = top2[sel]
        tvr = tv[sel]
        own = np.where(row[:, 0] == e, tvr[:, 0], tvr[:, 1])
        other = np.where(row[:, 0] == e, tvr[:, 1], tvr[:, 0])

        xt = np.zeros((C, cap), dtype=ml_dtypes.bfloat16)
        xt[:, :cnt] = xf[sel].T.astype(ml_dtypes.bfloat16)
        dlg = np.full((cap,), -60.0, dtype=np.float32)
        dlg[:cnt] = own - other
        dlg_b = np.ascontiguousarray(
            np.broadcast_to(dlg[None, :], (P, cap)), dtype=np.float32
        )
        w2h = np.ascontiguousarray(
            W2[e].reshape(F // P, P, C // P, P).transpose(1, 2, 0, 3)
        ).astype(ml_dtypes.bfloat16)
        in_maps.append(
            {
                "xt": xt,
                "w1": W1[e].astype(ml_dtypes.bfloat16),
                "w2": w2h,
                "dlg": dlg_b,
            }
        )

    # ---- Expert FFN on the 8 NeuronCores ----
    nc = _nc_cache.get(cap)
    if nc is None:
        nc = _nc_cache[cap] = _build_ffn(cap)
    res = run_bass_kernel_spmd(nc, in_maps, core_ids=list(range(E)))

    global last_run_info
    last_run_info = {
        "cap": cap,
        "counts": counts,
        "exec_time_ns": res.exec_time_ns,
        "mean_exec_time_ns": res.mean_exec_time_ns,
        "instructions_and_trace": res.instructions_and_trace,
        "profile_json": res.profile_json,
    }

    # ---- Combine (weighted scatter-add) ----
    out = np.zeros((N, C), dtype=np.float32)
    for e in range(E):
        sel = sels[e]
        out[sel] += res.results[e]["yt"][:, : len(sel)].T
    return out.reshape(B, T, C)


# revision 23
# speedup vs baseline: 1.0356x; 1.0022x over previous
# MoE layer (8 experts, top-2) on 8 TRN2 NeuronCores.
#
# Strategy: expert parallelism (core e owns expert e), per the sharding hint.
#   * Host (control plane): computes gate routing decisions, dispatches
#     ("all-to-all") each token's row to the core(s) owning its top-2 experts,
#     and combines the per-expert partial outputs back into the full output.
#   * Device (data plane): for each core e, computes
#         yT = sigmoid(dlg) * ( W2[e].T @ gelu( W1[e].T @ xT ) )
#     where xT is the (C x CAP) gathered token block for expert e (transposed
#     so the contraction dim lives on SBUF partitions), and sigmoid(dlg) is
#     exactly the top-2 softmax weight for the owning expert
#     (softmax([a,b])[0] == sigmoid(a-b)).
#
# Dataflow is fully transposed (features on partitions, tokens on the moving
# free dim) so neither matmul needs an intermediate transpose:
#     phase 1:  hT(F x T)  = W1.T @ xT   (accumulate over C tiles)  -> gelu
#     phase 2:  yT(C x T)  = W2.T @ hT   (accumulate over F tiles)  -> * ce
# W1 stays resident in SBUF in bf16; W2 streams per token block (its reloads
# hide under the PE-bound compute); tokens stream in blocks of 512 (the
# PSUM-bank moving-dim limit for fp32 accumulation).

import math

import numpy as np
import ml_dtypes

import concourse.bass as bass
import concourse.mybir as mybir
import concourse.tile as tile
from concourse import bacc
from concourse.bass_utils import run_bass_kernel_spmd

C = 1024          # d_model
F = 4096          # d_ff
E = 8             # experts == cores
P = 128           # SBUF partitions
NTOK = 512        # moving-dim token block (one PSUM bank of fp32)
BF16 = mybir.dt.bfloat16
F32 = mybir.dt.float32

# Filled by kernel() on each call, for the test harness to inspect.
last_run_info: dict = {}

# NEFF-module memo: cap -> compiled Bass module (routing is deterministic in
# the inputs, so repeat calls reuse the same module and its cached NEFF).
_nc_cache: dict = {}


def _build_ffn(cap: int, act_fn=None, ntok: int = NTOK) -> bass.Bass:
    """Per-core expert-FFN kernel: yt = sigmoid(dlg) * (w2.T @ gelu(w1.T @ xt))."""
    if act_fn is None:
        act_fn = mybir.ActivationFunctionType.Gelu
    nc = bacc.Bacc()
    CO = C // P   # 8 c-tiles
    FO = F // P   # 32 f-tiles

    xt = nc.dram_tensor("xt", [C, cap], BF16, kind="ExternalInput")
    w1 = nc.dram_tensor("w1", [C, F], BF16, kind="ExternalInput")
    # w2 is host-rearranged to [fi, co, fo, cc] so each (co) chunk streams as
    # one contiguous 8 KiB-per-partition DMA.
    w2 = nc.dram_tensor("w2", [P, CO, FO, P], BF16, kind="ExternalInput")
    dlg = nc.dram_tensor("dlg", [P, cap], F32, kind="ExternalInput")
    yt = nc.dram_tensor("yt", [C, cap], F32, kind="ExternalOutput")

    xt_r = xt.rearrange("(co ci) t -> ci co t", ci=P)
    yt_r = yt.rearrange("(co ci) t -> ci co t", ci=P)
    w1_r = w1.rearrange("(co ci) f -> ci co f", ci=P)

    with tile.TileContext(nc) as tc:
        with (
            tc.tile_pool(name="wts", bufs=1) as wpool,
            tc.tile_pool(name="w2s", bufs=3) as w2pool,
            tc.tile_pool(name="xts", bufs=2) as xpool,
            tc.tile_pool(name="hts", bufs=1) as hpool,
            tc.tile_pool(name="ces", bufs=2) as cepool,
            tc.tile_pool(name="yts", bufs=3) as ypool,
            tc.tile_pool(name="ps", bufs=4, space="PSUM") as pspool,
        ):
            # Block 0's token DMAs are issued BEFORE the w1 load: the DMA
            # queue is FIFO, and the first matmul needs xt — queueing 8 MiB
            # of w1 ahead of it costs a ~23 us PE ramp (measured in the
            # cost-model timeline).
            xt0 = xpool.tile([P, CO, ntok], BF16, tag="xt")
            nc.sync.dma_start(xt0[:, :, : min(ntok, cap)], xt_r[:, :, : min(ntok, cap)])

            # Resident w1 (bf16, 64 KiB/partition), loaded in f-major chunks
            # so phase 1's fo-th psum group only waits for the chunk covering
            # it, not the whole 8 MiB. w2 streams per token block.
            w1_sb = wpool.tile([P, CO, F], BF16, tag="w1")
            FCH = 1024
            for f0 in range(0, F, FCH):
                for co in range(CO):
                    nc.sync.dma_start(
                        w1_sb[:, co, f0 : f0 + FCH], w1_r[:, co, f0 : f0 + FCH]
                    )

            nblk = (cap + ntok - 1) // ntok
            for b in range(nblk):
                t0 = b * ntok
                tn = min(ntok, cap - t0)

                if b == 0:
                    xt_t = xt0
                else:
                    xt_t = xpool.tile([P, CO, ntok], BF16, tag="xt")
                    nc.sync.dma_start(xt_t[:, :, :tn], xt_r[:, :, t0 : t0 + tn])
                # Combine weight ce = sigmoid(dlg) = 0.5*tanh(dlg/2) + 0.5
                # (tanh shares an ACT table with gelu; sigmoid does not).
                # dlg isn't needed until phase 2, so even block 0's load sits
                # after the w1 chunks without stalling anything.
                dlg_t = cepool.tile([P, ntok], F32, tag="dlg")
                nc.sync.dma_start(dlg_t[:, :tn], dlg[:, t0 : t0 + tn])
                ce_t = cepool.tile([P, ntok], F32, tag="ce")
                nc.scalar.activation(
                    ce_t[:, :tn], dlg_t[:, :tn],
                    mybir.ActivationFunctionType.Tanh, scale=0.5,
                )
                nc.vector.tensor_scalar(
                    ce_t[:, :tn], ce_t[:, :tn], 0.5, 0.5,
                    mybir.AluOpType.mult, mybir.AluOpType.add,
                )

                # Phase 1: hT = gelu(W1.T @ xT) for this token block.
                ht_t = hpool.tile([P, FO, ntok], BF16, tag="ht")
                for fo in range(FO):
                    ps = pspool.tile([P, ntok], F32, tag="ps")
                    for co in range(CO):
                        nc.tensor.matmul(
                            ps[:, :tn],
                            w1_sb[:, co, fo * P : (fo + 1) * P],
                            xt_t[:, co, :tn],
                            start=(co == 0),
                            stop=(co == CO - 1),
                        )
                    nc.scalar.activation(ht_t[:, fo, :tn], ps[:, :tn], act_fn)

                # Phase 2: yT = ce * (W2.T @ hT).
                for co in range(CO):
                    w2_t = w2pool.tile([P, FO, P], BF16, tag="w2s")
                    nc.sync.dma_start(w2_t[:], w2[:, co, :, :])
                    ps2 = pspool.tile([P, ntok], F32, tag="ps")
                    for fo in range(FO):
                        nc.tensor.matmul(
                            ps2[:, :tn],
                            w2_t[:, fo, :],
                            ht_t[:, fo, :tn],
                            start=(fo == 0),
                            stop=(fo == FO - 1),
                        )
                    y_t = ypool.tile([P, ntok], F32, tag="y")
                    nc.vector.tensor_tensor(
                        y_t[:, :tn], ps2[:, :tn], ce_t[:, :tn],
                        mybir.AluOpType.mult,
                    )
                    nc.sync.dma_start(yt_r[:, co, t0 : t0 + tn], y_t[:, :tn])

    # bacc passes: register allocation, and crucially generate_event_semaphores,
    # which splits multi-wait sync conditions (HW allows 1 wait per instruction).
    nc.compile()

    # Guard: the Tile allocator believes SBUF is 224 KiB/partition (the ISA
    # constant), but exceeding ~192 KiB crashes the TRN2 exec unit. Keep a
    # hard ceiling so overflows fail at build time, not on silicon.
    hw = 0
    for alloc in nc.to_json()["functions"][0]["allocations"]:
        for ml in alloc.get("memorylocations") or []:
            if ml.get("type") == "SB":
                hw = max(hw, ml["addr"] + ml["dims"][1])
    assert hw <= 184 * 1024, f"SBUF high-water {hw / 1024:.1f} KiB exceeds 184 KiB"
    return nc


def _gate_jax_cpu(xf: np.ndarray, Wg: np.ndarray):
    """Reproduce the reference's gate bit-exactly: fp32 matmul + lax.top_k
    on the jax CPU backend (including its tie-breaking). Falls back to a
    numpy gate (correct except possibly on exact fp32 knife-edge ties) if
    jax is unavailable."""
    try:
        import jax

        cpu = jax.devices("cpu")[0]
        with jax.default_device(cpu):
            logits = jax.device_put(xf, cpu) @ jax.device_put(Wg, cpu)
            tv, ti = jax.lax.top_k(logits, 2)
            return np.asarray(ti), np.asarray(tv)
    except Exception:
        logits = xf @ Wg
        part = np.argpartition(-logits, 1, axis=1)[:, :2]
        pv = np.take_along_axis(logits, part, axis=1)
        order = np.argsort(-pv, axis=1, kind="stable")
        ti = np.take_along_axis(part, order, axis=1)
        tv = np.take_along_axis(logits, ti, axis=1)
        return ti, tv


def kernel(x, Wg, W1, W2):
    x = np.asarray(x, dtype=np.float32)
    Wg = np.asarray(Wg, dtype=np.float32)
    W1 = np.asarray(W1, dtype=np.float32)
    W2 = np.asarray(W2, dtype=np.float32)

    B, T, _ = x.shape
    N = B * T
    xf = x.reshape(N, C)

    # ---- Gate + routing (control plane) ----
    # Routing decisions are knife-edge sensitive: for this problem one token
    # has a 2.7e-7 gap between its 2nd and 3rd expert logits, smaller than
    # fp32 GEMM rounding differences between BLAS implementations. Compute
    # the gate with the same jax-on-CPU ops the reference uses so the top-2
    # selection matches it bit-for-bit.
    top2, tv = _gate_jax_cpu(xf, Wg)                        # (N, 2) ids / logits

    sels = []
    counts = []
    for e in range(E):
        sel = np.nonzero((top2 == e).any(axis=1))[0]
        sels.append(sel)
        counts.append(len(sel))
    # cap needs no partition alignment — tokens are the free dim everywhere.
    # Round to even so bf16 rows stay 4-byte aligned.
    cap = max(NTOK, math.ceil(max(counts) / 2) * 2)

    # ---- Token dispatch (all-to-all equivalent) ----
    in_maps = []
    for e in range(E):
        sel = sels[e]
        cnt = len(sel)
        row = top2[sel]
        tvr = tv[sel]
        own = np.where(row[:, 0] == e, tvr[:, 0], tvr[:, 1])
        other = np.where(row[:, 0] == e, tvr[:, 1], tvr[:, 0])

        xt = np.zeros((C, cap), dtype=ml_dtypes.bfloat16)
        xt[:, :cnt] = xf[sel].T.astype(ml_dtypes.bfloat16)
        dlg = np.full((cap,), -60.0, dtype=np.float32)
        dlg[:cnt] = own - other
        dlg_b = np.ascontiguousarray(
            np.broadcast_to(dlg[None, :], (P, cap)), dtype=np.float32
        )
        w2h = np.ascontiguousarray(
            W2[e].reshape(F // P, P, C // P, P).transpose(1, 2, 0, 3)
        ).astype(ml_dtypes.bfloat16)
        in_maps.append(
            {
                "xt": xt,
                "w1": W1[e].astype(ml_dtypes.bfloat16),
                "w2": w2h,
                "dlg": dlg_b,
            }
        )

    # ---- Expert FFN on the 8 NeuronCores ----
    nc = _nc_cache.get(cap)
    if nc is None:
        nc = _nc_cache[cap] = _build_ffn(cap)
    res = run_bass_kernel_spmd(nc, in_maps, core_ids=list(range(E)))

    global last_run_info
    last_run_info = {
        "cap": cap,
        "counts": counts,
        "exec_time_ns": res.exec_time_ns,
        "mean_exec_time_ns": res.mean_exec_time_ns,
        "instructions_and_trace": res.instructions_and_trace,
        "profile_json": res.profile_json,
    }

    # ---- Combine (weighted scatter-add) ----
    out = np.zeros((N, C), dtype=np.float32)
    for e in range(E):
        sel = sels[e]
        out[sel] += res.results[e]["yt"][:, : len(sel)].T
    return out.reshape(B, T, C)
